# revision 2
# baseline (speedup 1.0000x reference)
"""Trainium2 Bass kernel for CubePadding (p=1) on x:[6,128,512,512] f32.

Sharding: channel dim C=128 split across 8 NeuronCores (16 channels each);
each core pads its slice independently (zero communication).

v5 design — PHASED load/store separation + clean HBM access only.

Measured on HW: concurrent HBM read+write caps at ~306 GB/s aggregate
while pure reads run 403 and pure writes 375 GB/s; small-element DRAM
writes (16B/4B pieces) poison a write phase far beyond their byte count
(HBM read-modify-write). So:

  - alternate pure-load / pure-store phases of G=3 channels (strict
    semaphore barrier; loads of group g+1 wait all stores of group g)
  - a tiny AllReduce per phase boundary keeps the 8 SPMD cores' phases
    aligned on the shared HBM (they otherwise drift 100+ us apart and
    one core's stores mix with another's loads); its ~15 us latency
    hides inside the preceding store phase
  - loads go to dense X tiles [128,4,512] (8KB descriptors; strided or
    64-partition layouts measured 25-100% slower); DVE (faces 0-2) and
    ACT (faces 3-5) relay X -> OT[c%3][f][128,4,514] at col offset 1,
    chasing the loads inside the load phase
  - stores are full 514-wide rows OT -> out[f,c,1:513,:] (8224B
    descriptors; halo columns ride along for free)
  - ALL top/bottom halo rows (12 kinds) are assembled in SBUF as
    [16,514] rows (16-partition slices of two [128,514] tiles),
    complete with corner pixels (DVE edge-column duplication), and
    stored as clean 2056B-descriptor row DMAs during store phases.
    Row->column transposes ride SBUF->SBUF scatter DMAs (no HBM);
    the only DRAM bounce left is the L1/R5 side-column prepass (2KB
    descriptors, read back during load phase 0).

HW: 712us (v2 mixed pipeline) -> this design targets ~550-580us.
"""

import os
import sys

import numpy as np

for _p in (
    "/root/.axon_site",
    "/root/.axon_site/_ro/trn_rl_repo",
    "/root/.axon_site/_ro/pypackages",
    "/opt/trn_rl_repo",
    "/opt/pypackages",
):
    if os.path.isdir(_p) and _p not in sys.path:
        sys.path.append(_p)

N_CORES = 8
FACES, C, H, W = 6, 128, 512, 512
CL = C // N_CORES  # channels per core


def build_nc_v5(cl=CL, h=H, w=W, G=3, x_slots=6, use_barrier=False):
    from contextlib import ExitStack

    from concourse import bass, mybir

    P = 128
    assert h % 4 == 0 and h // 4 == P
    f32 = mybir.dt.float32

    nc = bass.Bass()
    x = nc.declare_dram_parameter("x", [FACES, cl, h, w], f32, isOutput=False)
    out = nc.declare_dram_parameter(
        "out", [FACES, cl, h + 2, w + 2], f32, isOutput=True
    )
    scrA = nc.dram_tensor("scrA", [2, cl, w], f32)  # L1/R5 col bounce
    ccin = nc.dram_tensor("ccin", [1, 8], f32)      # barrier AllReduce bufs
    ccout = nc.dram_tensor("ccout", [1, 8], f32)

    # ---- SBUF ----
    X = [nc.alloc_sbuf_tensor(f"x{s}", [P, 4, w], f32) for s in range(x_slots)]
    OT = [
        [nc.alloc_sbuf_tensor(f"o{s}f{f}", [P, 4, w + 2], f32) for f in range(FACES)]
        for s in range(G)
    ]
    # COL[k][p, c, j]: side-col value at row 4p+j of channel c.
    # k: 0=L1 (x3 r511 rev)  1=R1 (x4 r511)  2=L5 (x3 r0)  3=R5 (x4 r0 rev)
    COL = [nc.alloc_sbuf_tensor(f"col{k}", [P, cl, 4], f32) for k in range(4)]
    # FO[i][k]: column extracts of channel i. k: 0=T3(x5c0) 1=T4(x5c511)
    # 2=D3(x1c0) 3=D4(x1c511)
    FO = [
        [nc.alloc_sbuf_tensor(f"fo{i}k{k}", [P, 4], f32) for k in range(4)]
        for i in range(cl)
    ]
    # Halo-row assembly tiles. 16-partition slices hold one [16,512/514]
    # row strip per kind (partition = channel).
    #   PREBIG:  0-15 T0src  16-31 T5src  32-47 D0src  48-63 D1src
    #            64-79 L1src 80-95 R5src  96-111 R1src 112-127 L5src
    #   ROWBIG1: 0-15 T0  16-31 T5  32-47 D0  48-63 D1
    #            64-79 T1  80-95 T2  96-111 D2  112-127 D5
    #   ROWBIG2: 0-15 T4  16-31 T3  32-47 D3  48-63 D4
    PREBIG = nc.alloc_sbuf_tensor("prebig", [P, w], f32)
    REVBIG = nc.alloc_sbuf_tensor("revbig", [P, w], f32)   # [64:96] used
    ROWBIG1 = nc.alloc_sbuf_tensor("rowbig1", [P, w + 2], f32)
    ROWBIG2 = nc.alloc_sbuf_tensor("rowbig2", [64, w + 2], f32)
    SRC2BIG = nc.alloc_sbuf_tensor("src2big", [64, w], f32)  # T4 0-15, D3 32-47

    groups = []
    c0 = 0
    while c0 < cl:
        g = min(G, cl - c0)
        groups.append(list(range(c0, c0 + g)))
        c0 += g
    cum = [0]
    for grp in groups:
        cum.append(cum[-1] + len(grp))

    # prepass load sources, PREBIG partition-slice order
    PRE_SRCS = [
        x[5][:, 0, :],       # T0src
        x[0][:, 0, :],       # T5src
        x[1][:, h - 1, :],   # D0src
        x[0][:, h - 1, :],   # D1src
        x[3][:, h - 1, :],   # L1src
        x[4][:, 0, :],       # R5src
        x[4][:, h - 1, :],   # R1src
        x[3][:, 0, :],       # L5src
    ]
    # direct rows into ROWBIG1[64:128]
    DROW_SRCS = [
        x[2][:, h - 1, :],   # T1
        x[5][:, h - 1, :],   # T2
        x[1][:, 0, :],       # D2
        x[2][:, 0, :],       # D5
    ]
    # ROWBIG1 row destinations (kind order matches partition slices)
    ROW1_DSTS = [
        out[0, :, 0, :],      # T0
        out[5, :, 0, :],      # T5
        out[0, :, h + 1, :],  # D0
        out[1, :, h + 1, :],  # D1
        out[1, :, 0, :],      # T1
        out[2, :, 0, :],      # T2
        out[2, :, h + 1, :],  # D2
        out[5, :, h + 1, :],  # D5
    ]
    ROW2_DSTS = [
        out[4, :, 0, :],      # T4
        out[3, :, 0, :],      # T3
        out[3, :, h + 1, :],  # D3
        out[4, :, h + 1, :],  # D4
    ]

    def col_view(ap):  # [cl, w] -> [P, cl, 4] walk order
        return ap.rearrange("c (p j) -> p c j", p=P)

    N_GP_OUT = 8 + 4  # ROWBIG1 + ROWBIG2 row stores

    with ExitStack() as stack:
        block = stack.enter_context(nc.Block())
        sem = lambda n: stack.enter_context(nc.semaphore(n))
        lsem = [sem(f"ls{f}") for f in range(FACES)]
        rel_dve = sem("rel_dve")
        rel_act = sem("rel_act")
        fill_a = sem("fill_a")
        fill_b = sem("fill_b")
        col_ready = sem("col_ready")
        pre_sem = sem("pre_sem")
        drow_sem = sem("drow_sem")
        rev_early = sem("rev_early")
        row1_ready = sem("row1_ready")
        scat_sem = sem("scat_sem")
        row2_ready = sem("row2_ready")
        scra_s = sem("scra_s")
        store_sem = sem("store_sem")
        gp_out = sem("gp_out")
        barrier_sem = sem("barrier_sem")

        def x_slot(i, f):
            return (6 * i + f) % x_slots

        @block.sync
        def _(sync: "bass.BassEngine"):
            done = 0
            for gi, grp in enumerate(groups):
                if gi > 0:
                    sync.wait_ge(store_sem, 16 * FACES * done)
                    if use_barrier:
                        # all cores finished their previous load phase
                        sync.wait_ge(barrier_sem, gi)
                for i in grp:
                    for f in range(FACES):
                        pos = 6 * i + f
                        if pos >= x_slots:
                            # slot reuse: relayout of its previous
                            # occupant (channel ip, face fp) done
                            ip, fp = divmod(pos - x_slots, 6)
                            if fp < 3:
                                sync.wait_ge(rel_dve, 3 * ip + fp + 1)
                            else:
                                sync.wait_ge(rel_act, 3 * ip + (fp - 3) + 1)
                        sync.dma_start(
                            out=X[x_slot(i, f)][:, :, :], in_=x[f, i]
                        ).then_inc(lsem[f], 16)
                done += len(grp)

        @block.vector
        def _(vector: "bass.BassEngine"):
            # prepass reversals: one DVE op covers T0/T5/D0/D1 into
            # ROWBIG1 (with their in-row reversal), one covers L1/R5
            vector.wait_ge(pre_sem, 16 * 8)
            vector.tensor_copy(
                ROWBIG1[0:64, 1 : w + 1], PREBIG[0:64, w - 1 :: -1]
            )
            vector.tensor_copy(
                REVBIG[64:96, :], PREBIG[64:96, w - 1 :: -1]
            ).then_inc(rev_early, 1)

            for gi, grp in enumerate(groups):
                # relayout chase + fill_a per channel
                for i in grp:
                    s = i % G
                    for f in (0, 1, 2):
                        vector.wait_ge(lsem[f], 16 * (i + 1))
                        vector.tensor_copy(
                            OT[s][f][:, :, 1 : w + 1], X[x_slot(i, f)][:, :, :]
                        ).then_inc(rel_dve, 1)
                    vector.wait_ge(rel_act, 3 * (i + 1))
                    # side cols of faces 0,2,3,4 from sibling OT edges
                    for df, dc, sf, sc in (
                        (0, 0, 4, w),      # L0 = x4 c511
                        (0, w + 1, 3, 1),  # R0 = x3 c0
                        (2, 0, 3, w),      # L2 = x3 c511
                        (2, w + 1, 4, 1),  # R2 = x4 c0
                        (3, 0, 0, w),      # L3 = x0 c511
                        (3, w + 1, 2, 1),  # R3 = x2 c0
                        (4, 0, 2, w),      # L4 = x2 c511
                        (4, w + 1, 0, 1),  # R4 = x0 c0
                    ):
                        vector.tensor_copy(
                            OT[s][df][:, :, dc], OT[s][sf][:, :, sc]
                        )
                    # column extracts for T3/T4/D3/D4
                    vector.tensor_copy(FO[i][0][:, :], OT[s][5][:, :, 1])
                    vector.tensor_copy(FO[i][1][:, :], OT[s][5][:, :, w])
                    vector.tensor_copy(FO[i][2][:, :], OT[s][1][:, :, 1])
                    vector.tensor_copy(
                        FO[i][3][:, :], OT[s][1][:, :, w]
                    ).then_inc(fill_a, 1)
                # fill_b per channel at group end (COL-prepass gated)
                for i in grp:
                    s = i % G
                    if i == 0:
                        vector.wait_ge(col_ready, 16 * 4)
                    vector.tensor_copy(OT[s][1][:, :, 0], COL[0][:, i, :])
                    vector.tensor_copy(OT[s][1][:, :, w + 1], COL[1][:, i, :])
                    vector.tensor_copy(OT[s][5][:, :, 0], COL[2][:, i, :])
                    vector.tensor_copy(
                        OT[s][5][:, :, w + 1], COL[3][:, i, :]
                    ).then_inc(fill_b, 1)
                if gi == 0:
                    # ROWBIG1 complete: add corner pixels (edge-column
                    # duplication across all 8 rows at once)
                    vector.wait_ge(drow_sem, 16 * 4)
                    vector.tensor_copy(ROWBIG1[:, 0:1], ROWBIG1[:, 1:2])
                    vector.tensor_copy(
                        ROWBIG1[:, w + 1 : w + 2], ROWBIG1[:, w : w + 1]
                    ).then_inc(row1_ready, 1)

            # tail: reverse bounced T4/D3 rows, add ROWBIG2 corners
            vector.wait_ge(scat_sem, 16 * 4 * cl)
            vector.tensor_copy(
                ROWBIG2[0:16, 1 : w + 1], SRC2BIG[0:16, w - 1 :: -1]
            )
            vector.tensor_copy(
                ROWBIG2[32:48, 1 : w + 1], SRC2BIG[32:48, w - 1 :: -1]
            )
            vector.tensor_copy(ROWBIG2[:, 0:1], ROWBIG2[:, 1:2])
            vector.tensor_copy(
                ROWBIG2[:, w + 1 : w + 2], ROWBIG2[:, w : w + 1]
            ).then_inc(row2_ready, 1)

        @block.scalar
        def _(scalar: "bass.BassEngine"):
            done = 0
            for gi, grp in enumerate(groups):
                # ACT relayout of faces 3-5, chasing the loads
                for i in grp:
                    s = i % G
                    for f in (3, 4, 5):
                        scalar.wait_ge(lsem[f], 16 * (i + 1))
                        scalar.copy(
                            OT[s][f][:, :, 1 : w + 1], X[x_slot(i, f)][:, :, :]
                        ).then_inc(rel_act, 1)
                done += len(grp)
                # store phase: strict barrier — every load of this group
                # complete (keeps HBM reads and writes separated)
                for f in range(FACES):
                    scalar.wait_ge(lsem[f], 16 * done)
                for i in grp:
                    s = i % G
                    scalar.wait_ge(fill_a, i + 1)
                    for f in (0, 2, 3, 4):
                        scalar.dma_start(
                            out=out[f, i, 1 : h + 1, :], in_=OT[s][f][:, :, :]
                        ).then_inc(store_sem, 16)
                for i in grp:
                    s = i % G
                    scalar.wait_ge(fill_b, i + 1)
                    for f in (1, 5):
                        scalar.dma_start(
                            out=out[f, i, 1 : h + 1, :], in_=OT[s][f][:, :, :]
                        ).then_inc(store_sem, 16)
            # final barrier: every output write complete
            scalar.wait_ge(store_sem, 16 * FACES * cl)
            scalar.wait_ge(gp_out, 16 * N_GP_OUT)

        @block.gpsimd
        def _(gpsimd: "bass.BassEngine"):
            # prepass loads (reads; overlap load phase 0)
            for k, src in enumerate(PRE_SRCS):
                gpsimd.dma_start(
                    out=PREBIG[16 * k : 16 * (k + 1), :], in_=src
                ).then_inc(pre_sem, 16)
            for j, src in enumerate(DROW_SRCS):
                gpsimd.dma_start(
                    out=ROWBIG1[64 + 16 * j : 80 + 16 * j, 1 : w + 1], in_=src
                ).then_inc(drow_sem, 16)
            # COL direct readbacks (reads)
            gpsimd.dma_start(
                out=COL[1][:, :, :], in_=col_view(x[4][:, h - 1, :])
            ).then_inc(col_ready, 16)
            gpsimd.dma_start(
                out=COL[2][:, :, :], in_=col_view(x[3][:, 0, :])
            ).then_inc(col_ready, 16)
            # bounce reversed L1/R5 through scrA (2KB descriptors)
            gpsimd.wait_ge(rev_early, 1)
            gpsimd.dma_start(out=scrA[0], in_=REVBIG[64:80, :]).then_inc(
                scra_s, 16
            )
            gpsimd.dma_start(out=scrA[1], in_=REVBIG[80:96, :]).then_inc(
                scra_s, 16
            )
            gpsimd.wait_ge(scra_s, 16 * 2)
            gpsimd.dma_start(
                out=COL[0][:, :, :], in_=col_view(scrA[0])
            ).then_inc(col_ready, 16)
            gpsimd.dma_start(
                out=COL[3][:, :, :], in_=col_view(scrA[1])
            ).then_inc(col_ready, 16)

            def phase_barrier(g):
                if not use_barrier:
                    return
                for f in range(FACES):
                    gpsimd.wait_ge(lsem[f], 16 * cum[g + 1])
                gpsimd.collective_compute(
                    "AllReduce",
                    mybir.AluOpType.add,
                    replica_groups=[list(range(N_CORES))],
                    ins=[ccin[:, :]],
                    outs=[ccout[:, :]],
                ).then_inc(barrier_sem, 1)

            def fo_channel(i):
                # T3/T4/D3/D4: SBUF->SBUF partition-transpose scatters
                gpsimd.wait_ge(fill_a, i + 1)
                gpsimd.dma_start(
                    out=ROWBIG2[16 + i : 17 + i, 1 : w + 1], in_=FO[i][0][:, :]
                ).then_inc(scat_sem, 16)  # T3
                gpsimd.dma_start(
                    out=SRC2BIG[i : i + 1, :], in_=FO[i][1][:, :]
                ).then_inc(scat_sem, 16)  # T4 (reversal pending)
                gpsimd.dma_start(
                    out=SRC2BIG[32 + i : 33 + i, :], in_=FO[i][2][:, :]
                ).then_inc(scat_sem, 16)  # D3 (reversal pending)
                gpsimd.dma_start(
                    out=ROWBIG2[48 + i : 49 + i, 1 : w + 1], in_=FO[i][3][:, :]
                ).then_inc(scat_sem, 16)  # D4

            # cycle 0
            phase_barrier(0)
            gpsimd.wait_ge(row1_ready, 1)
            for k, dst in enumerate(ROW1_DSTS):
                gpsimd.dma_start(
                    out=dst, in_=ROWBIG1[16 * k : 16 * (k + 1), :]
                ).then_inc(gp_out, 16)
            for i in groups[0]:
                fo_channel(i)
            # remaining cycles
            for gi in range(1, len(groups)):
                if gi < len(groups) - 1:
                    phase_barrier(gi)
                for i in groups[gi]:
                    fo_channel(i)
            # tail: ROWBIG2 row stores
            gpsimd.wait_ge(row2_ready, 1)
            for k, dst in enumerate(ROW2_DSTS):
                gpsimd.dma_start(
                    out=dst, in_=ROWBIG2[16 * k : 16 * (k + 1), :]
                ).then_inc(gp_out, 16)

    return nc


_built_nc = None

TRACE = False
LAST_RESULTS = None


def kernel(x, lrtd_pad):
    global _built_nc, LAST_RESULTS
    p = int(lrtd_pad)
    assert p == 1, f"kernel hardcodes p=1, got {p}"
    x = np.asarray(x, dtype=np.float32)
    assert x.shape == (FACES, C, H, W), x.shape

    from concourse.bass_utils import run_bass_kernel_spmd

    if _built_nc is None:
        _built_nc = build_nc_v5()

    in_maps = [
        {"x": np.ascontiguousarray(x[:, i * CL : (i + 1) * CL])}
        for i in range(N_CORES)
    ]
    res = run_bass_kernel_spmd(
        _built_nc, in_maps, list(range(N_CORES)), trace=TRACE
    )
    LAST_RESULTS = res
    return np.concatenate([r["out"] for r in res.results], axis=1)
